# revision 40
# baseline (speedup 1.0000x reference)
"""Trainium2 Bass kernel for a dense transformer block, distributed over 8
NeuronCores.

Sharding:
  phase 1 (attention): tensor-parallel over heads — each core computes 2 of
    the 16 heads end-to-end (QKV projections + causal softmax(QK^T)V), and
    returns the unnormalized per-head output O^T together with the softmax
    denominators (obtained via a ones-column appended to V).
  phase 2 (Wo + norms + FFN): data-parallel over tokens — each core handles
    512 of the 4096 token rows with replicated weights.

v4 notes:
  - All DRAM inputs are host-packed so every DMA reads a contiguous region
    (>=2KB per partition line); DMAs are spread over the sync/scalar/vector
    queues to use multiple DMA rings in parallel.
  - Phase 1 interleaves batch-1 QKV work as PE filler into batch-0's
    exp-bound stripe phase (keeps the PE HAM clock-gate warm); batch 1's
    stripe phase merges both heads into one PSUM tile with a single wide
    exp per stripe to cut scalar-engine overhead.
  - Phase 2: Wo stage streams against the DMA with 3 rotating PSUM slots;
    residuals are injected into PSUM via identity/ones matmuls; silu runs
    directly on the scalar engine; the last 8 W2 d-chunks are staggered per
    token block so outputs drain incrementally; outputs are bf16.
"""

import math
from contextlib import ExitStack

import ml_dtypes
import numpy as np

BF_NP = ml_dtypes.bfloat16

import concourse.bass as bass
import concourse.mybir as mybir
import concourse.tile as tile
from concourse import bacc
from concourse.bass_utils import run_bass_kernel_spmd
from concourse.masks import make_identity

FP = mybir.dt.float32
BF = mybir.dt.bfloat16
F8 = mybir.dt.float8e4
F8_NP = ml_dtypes.float8_e4m3fn
AF = mybir.ActivationFunctionType

N_CORES = 8
P = 128
EPS = 1e-6

LAST_EXEC_NS = []
LAST_TRACES = []


# --------------------------------------------------------------------------
# phase 1: per-core attention over a pair of heads
# --------------------------------------------------------------------------

def build_phase1(B, T, C, DH):
    HP = 2                      # heads per core
    DA = DH + 1                 # head dim + ones row (softmax denominator)
    NCC = C // P                # contraction chunks (8)
    NCP = NCC // 2              # contraction chunk pairs (4, fp8 DoubleRow)
    NT = T // P                 # 128-token blocks (16)
    NQT = T // 1024             # 1024-wide q tiles per batch (2)
    DW = HP * DH                # packed head dims (128)
    # NOTE: reference scales by C**-0.5 (not DH); q/k/v weights are scaled
    # x32 host-side for fp8 range, so fold 1/(32*32) into the exp scale
    scale = float(C) ** -0.5 / 1024.0
    DR = mybir.MatmulPerfMode.DoubleRow

    nc = bacc.Bacc("TRN2", debug=False)
    # host-packed fp8 layouts (contiguous per DMA)
    xT_d = nc.dram_tensor("xT", [B, NCP, P, 2, T], F8,
                          kind="ExternalInput").ap()
    wq_d = nc.dram_tensor("wq", [P, NCP, 2, DW], F8, kind="ExternalInput").ap()
    wk_d = nc.dram_tensor("wk", [P, NCP, 2, DW], F8, kind="ExternalInput").ap()
    wv_d = nc.dram_tensor("wv", [P, NCP, 2, DW], F8, kind="ExternalInput").ap()
    ot_d = nc.dram_tensor("ot", [B, HP, DA, T], BF, kind="ExternalOutput").ap()

    with tile.TileContext(nc) as tc, ExitStack() as ctx:
        const = ctx.enter_context(tc.tile_pool(name="const", bufs=1))
        xpool = ctx.enter_context(tc.tile_pool(name="xp", bufs=1))
        wpool = ctx.enter_context(tc.tile_pool(name="wp", bufs=1))
        qk_pool = ctx.enter_context(tc.tile_pool(name="qk", bufs=1))
        vaug_pool = ctx.enter_context(tc.tile_pool(name="vaug", bufs=1))
        vt_pool = ctx.enter_context(tc.tile_pool(name="vtp", bufs=2))
        pt_pool = ctx.enter_context(tc.tile_pool(name="pt", bufs=3))
        ot_pool = ctx.enter_context(tc.tile_pool(name="otp", bufs=2))

        # weights + x DMAs, spread across queues, in consumption order
        xts = [[None] * NCP for _ in range(B)]
        wtile = {}
        wtile["q"] = wpool.tile([P, NCP, 2, DW], F8, tag="wq", name="wq")
        nc.sync.dma_start(out=wtile["q"][:], in_=wq_d)
        wtile["k"] = wpool.tile([P, NCP, 2, DW], F8, tag="wk", name="wk")
        nc.scalar.dma_start(out=wtile["k"][:], in_=wk_d)
        for b in range(B):
            for cp in range(NCP):
                t = xpool.tile([P, 2, T], F8, tag=f"x{b}_{cp}")
                eng = nc.sync if cp % 2 == 0 else nc.scalar
                eng.dma_start(out=t[:], in_=xT_d[b, cp])
                xts[b][cp] = t
        wtile["v"] = wpool.tile([P, NCP, 2, DW], F8, tag="wv", name="wv")
        nc.gpsimd.dma_start(out=wtile["v"][:], in_=wv_d)

        # warm the exp table during the DMA stall
        junk = const.tile([P, 1], FP)
        nc.vector.memset(junk[:], 0.0)
        junk2 = const.tile([P, 1], FP)
        nc.scalar.activation(junk2[:], junk[:], AF.Exp)

        negmask = const.tile([P, P], FP)
        nc.gpsimd.memset(negmask[:], 0.0)
        nc.gpsimd.affine_select(
            out=negmask[:], in_=negmask[:],
            compare_op=mybir.AluOpType.is_ge, fill=-1e30,
            base=0, pattern=[[1, P]], channel_multiplier=-1)
        ident = const.tile([P, P], BF)
        make_identity(nc, ident[:])
        ones_col = const.tile([P, HP, NT, 1], FP)
        nc.vector.memset(ones_col[:], 1.0)

        qts, kts, vaugs = [], [], []
        for b in range(B):
            qts.append(qk_pool.tile([P, T], BF, tag=f"qt{b}", name=f"qt{b}"))
            kts.append(qk_pool.tile([P, T], BF, tag=f"kt{b}", name=f"kt{b}"))
            # head-major so stripe pairs are adjacent: [P, HP, NT, DA]
            va = vaug_pool.tile([P, HP, NT, DA], BF, tag=f"va{b}",
                                name=f"va{b}")
            nc.vector.tensor_copy(va[:, :, :, DA - 1:DA], ones_col[:])
            vaugs.append(va)

        # ---- stage A(b0): q/k projections, cp-outer against the DMA stream
        b0, b1 = 0, 1
        with tc.tile_pool(name="qk_ps", bufs=1, space="PSUM") as qk_ps:
            q_ps = qk_ps.tile([P, T], FP, tag="qps", name="qps")
            k_ps = qk_ps.tile([P, T], FP, tag="kps", name="kps")
            for cp in range(NCP):
                for wname, ps in (("q", q_ps), ("k", k_ps)):
                    for n in range(T // 512):
                        nc.tensor.matmul(
                            ps[:, n * 512:(n + 1) * 512],
                            wtile[wname][:, cp, :, :],
                            xts[b0][cp][:, :, n * 512:(n + 1) * 512],
                            start=(cp == 0), stop=(cp == NCP - 1),
                            perf_mode=DR)
            for half in range(T // 1024):
                sl = slice(half * 1024, (half + 1) * 1024)
                nc.vector.tensor_copy(qts[b0][:, sl], q_ps[:, sl])
                nc.vector.tensor_copy(kts[b0][:, sl], k_ps[:, sl])

        # ---- filler quanta (run inside b0's stripe sections)
        fl_ctx = ExitStack()
        fl_ps = fl_ctx.enter_context(
            tc.tile_pool(name="fl_ps", bufs=1, space="PSUM"))

        def v_quant(b, n8):
            vps = fl_ps.tile([P, 512], FP, tag="proj", name="proj")
            for cp in range(NCP):
                nc.tensor.matmul(
                    vps[:], wtile["v"][:, cp, :, :],
                    xts[b][cp][:, :, n8 * 512:(n8 + 1) * 512],
                    start=(cp == 0), stop=(cp == NCP - 1), perf_mode=DR)
            vt = vt_pool.tile([P, 512], BF, tag="vt")
            nc.vector.tensor_copy(vt[:], vps[:])
            tp = fl_ps.tile([P, 4, P], BF, tag="tp", name="tp")
            for u in range(4):
                nc.tensor.transpose(
                    tp[:, u, :], vt[:, u * P:(u + 1) * P], ident[:])
            for h in range(HP):
                nc.vector.tensor_copy(
                    vaugs[b][:, h, n8 * 4:(n8 + 1) * 4, 0:DH],
                    tp[:, :, h * DH:(h + 1) * DH])

        def qk_quant(b, name, n8):
            dst = qts[b] if name == "q" else kts[b]
            ps = fl_ps.tile([P, 512], FP, tag="proj", name="proj")
            for cp in range(NCP):
                nc.tensor.matmul(
                    ps[:], wtile[name][:, cp, :, :],
                    xts[b][cp][:, :, n8 * 512:(n8 + 1) * 512],
                    start=(cp == 0), stop=(cp == NCP - 1), perf_mode=DR)
            nc.vector.tensor_copy(dst[:, n8 * 512:(n8 + 1) * 512], ps[:])

        v_quant(b0, 0)
        v_quant(b0, 1)
        filler = [lambda: v_quant(b0, 2), lambda: v_quant(b0, 3)]
        for n8 in (0, 1):
            filler.append(lambda n=n8: qk_quant(b1, "k", n))
        for n8 in (0, 1):
            filler.append(lambda n=n8: qk_quant(b1, "q", n))
        for n8 in (0, 1):
            filler.append(lambda n=n8: v_quant(b1, n))
        for n8 in (2, 3):
            filler.append(lambda n=n8: qk_quant(b1, "k", n))
        for n8 in (2, 3):
            filler.append(lambda n=n8: qk_quant(b1, "q", n))
        for n8 in (2, 3):
            filler.append(lambda n=n8: v_quant(b1, n))

        def chunks_for(a0, q_hi):
            out = []
            for m in range(a0 // 512, q_hi // 512):
                out.append((max(a0, m * 512), (m + 1) * 512))
            return out

        def section(b, h, kq, s_ps, o_ps, use_filler):
            q_lo, q_hi = 1024 * kq, 1024 * (kq + 1)
            hs = slice(h * DH, (h + 1) * DH)
            o_t = o_ps.tile([DA, 1024], FP, tag="o", name="o")
            pend = None

            def emit_pv(ent):
                j, chunks, ptk = ent
                va = vaugs[b][:, h, j, :]
                for (a, e) in chunks:
                    nc.tensor.matmul(
                        o_t[:, a - q_lo:e - q_lo],
                        va, ptk[:, a - q_lo:e - q_lo],
                        start=(j == 0), stop=(j == e // P - 1))

            for j in range(8 * (kq + 1)):
                s0 = j * P
                a0 = max(s0, q_lo)
                chunks = chunks_for(a0, q_hi)
                stl = s_ps.tile([P, 1024], FP, tag="s", name="s")
                for (a, e) in chunks:
                    nc.tensor.matmul(
                        stl[:, a - q_lo:e - q_lo],
                        kts[b][hs, s0:s0 + P], qts[b][hs, a:e],
                        start=True, stop=True)
                if q_lo <= s0:
                    nc.vector.tensor_add(
                        stl[:, s0 - q_lo:s0 - q_lo + P],
                        stl[:, s0 - q_lo:s0 - q_lo + P], negmask[:])
                ptk = pt_pool.tile([P, 1024], BF, tag="ptk")
                nc.scalar.activation(
                    ptk[:, a0 - q_lo:1024], stl[:, a0 - q_lo:1024],
                    AF.Exp, scale=scale)
                if pend is not None:
                    emit_pv(pend)
                pend = (j, chunks, ptk)
                if use_filler and j % 2 == 0 and filler:
                    filler.pop(0)()
            emit_pv(pend)
            osb = ot_pool.tile([DA, 1024], BF, tag="osb")
            nc.vector.tensor_copy(osb[:], o_t[:])
            nc.gpsimd.dma_start(out=ot_d[b, h, :, q_lo:q_hi], in_=osb[:])

        with tc.tile_pool(name="s_ps", bufs=2, space="PSUM") as s_ps, \
             tc.tile_pool(name="o_ps", bufs=1, space="PSUM") as o_ps:
            for (h, kq) in ((0, 0), (0, 1), (1, 0), (1, 1)):
                section(b0, h, kq, s_ps, o_ps, True)
        while filler:
            filler.pop(0)()
        fl_ctx.close()
        with tc.tile_pool(name="s_ps1", bufs=3, space="PSUM") as s_ps1, \
             tc.tile_pool(name="o_ps1", bufs=1, space="PSUM") as o_ps1:
            for (h, kq) in ((0, 0), (0, 1), (1, 0), (1, 1)):
                section(b1, h, kq, s_ps1, o_ps1, False)
    nc.compile()
    return nc


# --------------------------------------------------------------------------
# phase 2: per-core Wo projection + residual + rmsnorm + FFN + rmsnorm
# --------------------------------------------------------------------------

def build_phase2(NTOK, C, DFF):
    NTB = NTOK // P             # 4 token blocks
    NCH = C // P                # 8 channel chunks
    NDF = DFF // P              # 32 dff chunks
    NG = DFF // 512             # 8 w1 groups
    STAG = 8                    # staggered tail d-chunks per token block

    nc = bacc.Bacc("TRN2", debug=False)
    # host-packed layouts (contiguous per DMA)
    xc_d = nc.dram_tensor("xc", [P, NTB * C], BF, kind="ExternalInput").ap()
    at_d = nc.dram_tensor("attnT", [2, P, 4, NTOK], F8,
                          kind="ExternalInput").ap()
    wo_d = nc.dram_tensor("wo", [2, P, 4, C], F8, kind="ExternalInput").ap()
    w1_d = nc.dram_tensor("w1", [NG, P, NCH * 512], BF,
                          kind="ExternalInput").ap()
    w2_d = nc.dram_tensor("w2", [NDF // 4, P, 4 * C], BF,
                          kind="ExternalInput").ap()
    g1_d = nc.dram_tensor("g1", [C], FP, kind="ExternalInput").ap()
    g2_d = nc.dram_tensor("g2", [C], FP, kind="ExternalInput").ap()
    b1_d = nc.dram_tensor("b1", [DFF], FP, kind="ExternalInput").ap()
    b2_d = nc.dram_tensor("b2", [C], BF, kind="ExternalInput").ap()
    out_d = nc.dram_tensor("out", [NTOK, C], BF, kind="ExternalOutput").ap()

    def bcast_rows(src_ap, cols):
        return bass.AP(tensor=src_ap.tensor, offset=src_ap.offset,
                       ap=[[0, P], [1, cols]])

    halves = ((0, 512), (512, 512))

    with tile.TileContext(nc) as tc, ExitStack() as ctx:
        const = ctx.enter_context(tc.tile_pool(name="const", bufs=1))
        work = ctx.enter_context(tc.tile_pool(name="work", bufs=2))
        stats = ctx.enter_context(tc.tile_pool(name="stats", bufs=4))
        h_pool = ctx.enter_context(tc.tile_pool(name="hp", bufs=1))
        at_pool = ctx.enter_context(tc.tile_pool(name="atp", bufs=1))
        out_pool = ctx.enter_context(tc.tile_pool(name="outp", bufs=2))
        wo_po = ctx.enter_context(tc.tile_pool(name="wop", bufs=1))

        # input DMAs: att/w1 on sync, wo/xc/w2 on scalar, vectors on vector
        att_t, wot_t = [], []
        for g in range(2):
            at4 = wo_po.tile([P, 4, NTOK], F8, tag=f"at{g}", name=f"at{g}")
            nc.sync.dma_start(out=at4[:], in_=at_d[g])
            att_t.append(at4)
            wo4 = wo_po.tile([P, 4, C], F8, tag=f"wo{g}", name=f"wo{g}")
            nc.scalar.dma_start(out=wo4[:], in_=wo_d[g])
            wot_t.append(wo4)
        xc_t = wo_po.tile([P, NTB * C], BF, tag="xc", name="xc")
        nc.scalar.dma_start(out=xc_t[:], in_=xc_d)

        eps_t = const.tile([P, 1], FP)
        nc.vector.memset(eps_t[:], EPS)
        g1b = const.tile([P, C], FP)
        nc.gpsimd.dma_start(out=g1b[:], in_=bcast_rows(g1_d, C))
        g2b = const.tile([P, C], FP)
        nc.gpsimd.dma_start(out=g2b[:], in_=bcast_rows(g2_d, C))
        b2row = const.tile([1, C], BF)
        nc.gpsimd.dma_start(
            out=b2row[:],
            in_=bass.AP(tensor=b2_d.tensor, offset=b2_d.offset,
                        ap=[[0, 1], [1, C]]))
        ones1 = const.tile([1, P], BF)
        nc.vector.memset(ones1[:], 1.0)
        b1s = const.tile([P, NDF], FP)
        nc.gpsimd.dma_start(
            out=b1s[:],
            in_=bass.AP(tensor=b1_d.tensor, offset=b1_d.offset,
                        ap=[[1, P], [P, NDF]]))
        # w1 groups (sync queue), 3-deep window
        w1_po = ctx.enter_context(tc.tile_pool(name="w1p", bufs=3))
        w1g = []
        for g in range(NG):
            t = w1_po.tile([P, NCH * 512], BF, tag="w1g", name="w1g")
            nc.sync.dma_start(out=t[:], in_=w1_d[g])
            w1g.append(t)
        # w2 quads (scalar queue), 3-deep window
        w2_po = ctx.enter_context(tc.tile_pool(name="w2p", bufs=3))
        w2q = []
        for q in range(NDF // 4):
            t = w2_po.tile([P, 4 * C], BF, tag="w2q", name="w2q")
            nc.scalar.dma_start(out=t[:], in_=w2_d[q])
            w2q.append(t)

        def w2c(d, lo, hi):
            base = (d % 4) * C
            return w2q[d // 4][:, base + lo:base + hi]

        junk = const.tile([P, 1], FP)
        nc.vector.memset(junk[:], 1.0)
        junk2 = const.tile([P, 1], FP)
        nc.scalar.activation(junk2[:], junk[:], AF.Sqrt)
        ident = const.tile([P, P], BF)
        make_identity(nc, ident[:])

        hT = h_pool.tile([P, NCH, NTOK], BF, tag="hT")
        h_bfs = []

        # ---- stage 0: o = attn@Wo + x; rmsnorm per token block; -> hT
        with tc.tile_pool(name="o_ps", bufs=1, space="PSUM") as o_ps, \
             tc.tile_pool(name="t_ps", bufs=2, space="PSUM") as t_ps:
            for pair_i in range(NTB // 2):
                pair = (2 * pair_i, 2 * pair_i + 1)
                # 3 rotating tag slots so a fresh pair never waits on the
                # slowest token block of the previous pair
                sl_ids = [(2 * pair_i) % 3, (2 * pair_i + 1) % 3]
                o2 = [[o_ps.tile([P, 512], FP, tag=f"o2_{sl_ids[si]}_{hi}",
                                 name="o2")
                       for hi in range(2)] for si in range(2)]
                for cp in range(NCH // 2):
                    g, jj = cp // 2, (cp % 2) * 2
                    for si, tb in enumerate(pair):
                        for hi, (hst, hw) in enumerate(halves):
                            nc.tensor.matmul(
                                o2[si][hi][:],
                                att_t[g][:, jj:jj + 2, tb * P:(tb + 1) * P],
                                wot_t[g][:, jj:jj + 2, hst:hst + hw],
                                start=(cp == 0), stop=False,
                                perf_mode=mybir.MatmulPerfMode.DoubleRow)
                for si, tb in enumerate(pair):
                    for hi, (hst, hw) in enumerate(halves):
                        nc.tensor.matmul(
                            o2[si][hi][:], ident[:],
                            xc_t[:, tb * C + hst:tb * C + hst + hw],
                            start=False, stop=True)
                for si, tb in enumerate(pair):
                    sq = work.tile([P, 512], FP, tag="sq")
                    ss = [stats.tile([P, 1], FP, tag=f"ss{hi}",
                                     name=f"ss{hi}") for hi in range(2)]
                    for hi in range(2):
                        nc.scalar.activation(sq[:], o2[si][hi][:], AF.Square,
                                             accum_out=ss[hi][:])
                    nc.vector.tensor_add(ss[0][:], ss[0][:], ss[1][:])
                    rstd = stats.tile([P, 1], FP, tag="rstd")
                    nc.scalar.activation(rstd[:], ss[0][:], AF.Sqrt,
                                         scale=1.0 / C, bias=eps_t[:])
                    rinv = stats.tile([P, 1], FP, tag="rinv")
                    nc.vector.reciprocal(rinv[:], rstd[:])
                    h_bf = h_pool.tile([P, C], BF, tag=f"h{tb}")
                    for hi, (hst, hw) in enumerate(halves):
                        nc.vector.scalar_tensor_tensor(
                            h_bf[:, hst:hst + hw], o2[si][hi][:], rinv[:],
                            g1b[:, hst:hst + hw],
                            op0=mybir.AluOpType.mult, op1=mybir.AluOpType.mult)
                    h_bfs.append(h_bf)
                    for g4 in range(2):
                        tp = t_ps.tile([P, 4, P], BF, tag="tp")
                        for u in range(4):
                            nc.tensor.transpose(
                                tp[:, u, :],
                                h_bf[:, (g4 * 4 + u) * P:(g4 * 4 + u + 1) * P],
                                ident[:])
                        nc.vector.tensor_copy(
                            hT[:, g4 * 4:(g4 + 1) * 4, tb * P:(tb + 1) * P],
                            tp[:])

        # ---- stage 1: aT = silu(W1^T @ h^T + b1) via scalar-engine silu
        ats = []
        with tc.tile_pool(name="a_ps", bufs=2, space="PSUM") as a_ps:
            for g in range(NG):
                aps = [a_ps.tile([P, NTOK], FP, tag=f"a{u}", name=f"a{u}")
                       for u in range(4)]
                for c in range(NCH):
                    for u in range(4):
                        nc.tensor.matmul(
                            aps[u][:],
                            w1g[g][:, c * 512 + u * P:c * 512 + (u + 1) * P],
                            hT[:, c, :],
                            start=(c == 0), stop=(c == NCH - 1))
                for u in range(4):
                    d = 4 * g + u
                    at_t = at_pool.tile([P, NTOK], BF, tag=f"at{d}")
                    nc.scalar.activation(at_t[:], aps[u][:], AF.Silu,
                                         bias=b1s[:, d:d + 1], scale=1.0)
                    ats.append(at_t)

        # ---- stage 2: f = aT^T @ W2 + h + b2; rmsnorm + store per block
        with tc.tile_pool(name="f_ps", bufs=1, space="PSUM") as f_ps:
            f2 = [f_ps.tile([P, C], FP, tag=f"f{tb}", name=f"f{tb}")
                  for tb in range(NTB)]
            for tb in range(NTB):
                for (hst, hw) in halves:
                    nc.tensor.matmul(
                        f2[tb][:, hst:hst + hw], ident[:],
                        h_bfs[tb][:, hst:hst + hw],
                        start=True, stop=False)
                    nc.tensor.matmul(
                        f2[tb][:, hst:hst + hw], ones1[:],
                        b2row[:, hst:hst + hw],
                        start=False, stop=False)
            for d in range(NDF - STAG):
                for tb in range(NTB):
                    for (hst, hw) in halves:
                        nc.tensor.matmul(
                            f2[tb][:, hst:hst + hw],
                            ats[d][:, tb * P:(tb + 1) * P],
                            w2c(d, hst, hst + hw),
                            start=False, stop=False)
            for tb in range(NTB):
                for d in range(NDF - STAG, NDF):
                    for (hst, hw) in halves:
                        nc.tensor.matmul(
                            f2[tb][:, hst:hst + hw],
                            ats[d][:, tb * P:(tb + 1) * P],
                            w2c(d, hst, hst + hw),
                            start=False, stop=(d == NDF - 1))
                sq = work.tile([P, C], FP, tag="sq2")
                ssum = stats.tile([P, 1], FP, tag="ssum2")
                nc.scalar.activation(sq[:], f2[tb][:], AF.Square,
                                     accum_out=ssum[:])
                rstd = stats.tile([P, 1], FP, tag="rstd2")
                nc.scalar.activation(rstd[:], ssum[:], AF.Sqrt,
                                     scale=1.0 / C, bias=eps_t[:])
                rinv = stats.tile([P, 1], FP, tag="rinv2")
                nc.vector.reciprocal(rinv[:], rstd[:])
                o = out_pool.tile([P, C], BF, tag="outt")
                nc.vector.scalar_tensor_tensor(
                    o[:], f2[tb][:], rinv[:], g2b[:],
                    op0=mybir.AluOpType.mult, op1=mybir.AluOpType.mult)
                nc.gpsimd.dma_start(
                    out=out_d[tb * P:(tb + 1) * P, :], in_=o[:])
    nc.compile()
    return nc


# --------------------------------------------------------------------------
# host orchestration
# --------------------------------------------------------------------------

_CACHE = {}


def _phase1(B, T, C, DH):
    key = ("p1", B, T, C, DH)
    if key not in _CACHE:
        _CACHE[key] = build_phase1(B, T, C, DH)
    return _CACHE[key]


def _phase2(NTOK, C, DFF):
    key = ("p2", NTOK, C, DFF)
    if key not in _CACHE:
        _CACHE[key] = build_phase2(NTOK, C, DFF)
    return _CACHE[key]


def _run(nc, in_maps):
    import os
    trace = bool(os.environ.get("KERNEL_TRACE"))
    res = run_bass_kernel_spmd(nc, in_maps, core_ids=list(range(N_CORES)),
                               trace=trace)
    LAST_EXEC_NS.append(res.exec_time_ns)
    LAST_TRACES.append(res.instructions_and_trace)
    return res.results


def _pack_rows(a, nrow):
    """[R, W] -> [P, (R//P//nrow groups)...]: group rows so each DMA tile
    [P, nrow*W] is contiguous: out[g, p, i, :] = a[(g*nrow+i)*P + p, :]."""
    R, W = a.shape
    ng = R // (P * nrow)
    return np.ascontiguousarray(
        a.reshape(ng, nrow, P, W).transpose(0, 2, 1, 3).reshape(
            ng, P, nrow * W))


def kernel(x, Wq, Wk, Wv, Wo, bo, W1, b1, W2, b2, g1, g2):
    f32 = lambda a: np.ascontiguousarray(np.asarray(a), dtype=np.float32)
    x = f32(x)
    Wq, Wk, Wv, Wo, bo = f32(Wq), f32(Wk), f32(Wv), f32(Wo), f32(bo)
    W1, b1, W2, b2, g1, g2 = f32(W1), f32(b1), f32(W2), f32(b2), f32(g1), f32(g2)

    B, T, C = x.shape
    H, _, DH = Wq.shape
    HP = H // N_CORES
    DA = DH + 1
    NCC = C // P
    DW = HP * DH
    LAST_EXEC_NS.clear()
    LAST_TRACES.clear()

    # ---- phase 1
    nc1 = _phase1(B, T, C, DH)
    NCP = NCC // 2
    xT = x.transpose(0, 2, 1).astype(F8_NP)            # [B, C, T] fp8
    # pack x: [B, NCP, P, 2, T] with (b,cp,p,i,t) = xT[b, (2cp+i)P+p, t]
    xP = np.ascontiguousarray(
        xT.reshape(B, NCP, 2, P, T).transpose(0, 1, 3, 2, 4))
    in1 = []
    for i in range(N_CORES):
        ws = {}
        for nm, W_ in (("wq", Wq), ("wk", Wk), ("wv", Wv)):
            pw = W_[HP * i:HP * (i + 1)].transpose(1, 0, 2).reshape(C, DW)
            # x32 for fp8 range; [P, NCP, 2, DW], (p,cp,i,m)=pw[(2cp+i)P+p,m]
            ws[nm] = np.ascontiguousarray(
                (pw * 32.0).astype(F8_NP)
                .reshape(NCP, 2, P, DW).transpose(2, 0, 1, 3))
        in1.append({"xT": xP, **ws})
    res1 = _run(nc1, in1)

    attn = np.empty((B, T, C), np.float32)
    for i in range(N_CORES):
        ot = res1[i]["ot"].astype(np.float32)          # [B, HP, DA, T]
        o = ot[:, :, :DH, :]
        den = ot[:, :, DH, :]
        on = o / (32.0 * den[:, :, None, :])           # undo v x32 scale
        for hh in range(HP):
            hcol = (HP * i + hh) * DH
            attn[:, :, hcol:hcol + DH] = on[:, hh].transpose(0, 2, 1)

    # ---- phase 2
    NTOK = B * T // N_CORES
    DFF = W1.shape[1]
    NTB = NTOK // P
    nc2 = _phase2(NTOK, C, DFF)
    # rmsnorm is scale-invariant: attnT x8 and wo x32 go into fp8 range,
    # and the residual x picks up the matching x256
    xf = ((x.reshape(B * T, C) + bo) * 256.0).astype(BF_NP)
    af = attn.reshape(B * T, C) * 8.0
    NCH = C // P
    NG = DFF // 512
    # w1P[g][p, c*512+f] = W1[c*128+p, g*512+f]
    w1P = np.ascontiguousarray(
        W1.astype(BF_NP).reshape(NCH, P, NG, 512).transpose(2, 1, 0, 3)
        .reshape(NG, P, NCH * 512))
    w2P = _pack_rows(W2.astype(BF_NP), 4)              # [8, P, 4*C]
    in2 = []
    for k in range(N_CORES):
        sl = slice(k * NTOK, (k + 1) * NTOK)
        atT = np.ascontiguousarray(af[sl].T).astype(F8_NP)   # [C, NTOK]
        in2.append({
            "xc": _pack_rows(xf[sl], NTB)[0],
            "attnT": _pack_rows(atT, 4).reshape(2, P, 4, NTOK),
            "wo": _pack_rows((Wo * 32.0).astype(F8_NP), 4)
                  .reshape(2, P, 4, C),
            "w1": w1P, "w2": w2P,
            "g1": g1, "g2": g2, "b1": b1, "b2": b2.astype(BF_NP),
        })
    res2 = _run(nc2, in2)
    out = np.concatenate(
        [res2[k]["out"].astype(np.float32) for k in range(N_CORES)], axis=0)
    return out.reshape(B, T, C)


# revision 41
# speedup vs baseline: 1.0819x; 1.0819x over previous
"""Trainium2 Bass kernel for a dense transformer block, distributed over 8
NeuronCores.

Sharding:
  phase 1 (attention): tensor-parallel over heads — each core computes 2 of
    the 16 heads end-to-end (QKV projections + causal softmax(QK^T)V), and
    returns the unnormalized per-head output O^T together with the softmax
    denominators (obtained via a ones-column appended to V).
  phase 2 (Wo + norms + FFN): data-parallel over tokens — each core handles
    512 of the 4096 token rows with replicated weights.

v4 notes:
  - All DRAM inputs are host-packed so every DMA reads a contiguous region
    (>=2KB per partition line); DMAs are spread over the sync/scalar/vector
    queues to use multiple DMA rings in parallel.
  - Phase 1 interleaves batch-1 QKV work as PE filler into batch-0's
    exp-bound stripe phase (keeps the PE HAM clock-gate warm); batch 1's
    stripe phase merges both heads into one PSUM tile with a single wide
    exp per stripe to cut scalar-engine overhead.
  - Phase 2: Wo stage streams against the DMA with 3 rotating PSUM slots;
    residuals are injected into PSUM via identity/ones matmuls; silu runs
    directly on the scalar engine; the last 8 W2 d-chunks are staggered per
    token block so outputs drain incrementally; outputs are bf16.
"""

import math
from contextlib import ExitStack

import ml_dtypes
import numpy as np

BF_NP = ml_dtypes.bfloat16

import concourse.bass as bass
import concourse.mybir as mybir
import concourse.tile as tile
from concourse import bacc
from concourse.bass_utils import run_bass_kernel_spmd
from concourse.masks import make_identity

FP = mybir.dt.float32
BF = mybir.dt.bfloat16
F8 = mybir.dt.float8e4
F8_NP = ml_dtypes.float8_e4m3fn
AF = mybir.ActivationFunctionType

N_CORES = 8
P = 128
EPS = 1e-6

LAST_EXEC_NS = []
LAST_TRACES = []


# --------------------------------------------------------------------------
# phase 1: per-core attention over a pair of heads
# --------------------------------------------------------------------------

def build_phase1(B, T, C, DH):
    HP = 2                      # heads per core
    DA = DH + 1                 # head dim + ones row (softmax denominator)
    NCC = C // P                # contraction chunks (8)
    NCP = NCC // 2              # contraction chunk pairs (4, fp8 DoubleRow)
    NT = T // P                 # 128-token blocks (16)
    NQT = T // 1024             # 1024-wide q tiles per batch (2)
    DW = HP * DH                # packed head dims (128)
    # NOTE: reference scales by C**-0.5 (not DH); q/k/v weights are scaled
    # x32 host-side for fp8 range, so fold 1/(32*32) into the exp scale
    scale = float(C) ** -0.5 / 1024.0
    DR = mybir.MatmulPerfMode.DoubleRow

    nc = bacc.Bacc("TRN2", debug=False)
    # host-packed fp8 layouts (contiguous per DMA)
    xT_d = nc.dram_tensor("xT", [B, NCP, P, 2, T], F8,
                          kind="ExternalInput").ap()
    wq_d = nc.dram_tensor("wq", [P, NCP, 2, DW], F8, kind="ExternalInput").ap()
    wk_d = nc.dram_tensor("wk", [P, NCP, 2, DW], F8, kind="ExternalInput").ap()
    wv_d = nc.dram_tensor("wv", [P, NCP, 2, DW], F8, kind="ExternalInput").ap()
    ot_d = nc.dram_tensor("ot", [B, HP, DA, T], BF, kind="ExternalOutput").ap()

    with tile.TileContext(nc) as tc, ExitStack() as ctx:
        const = ctx.enter_context(tc.tile_pool(name="const", bufs=1))
        xpool = ctx.enter_context(tc.tile_pool(name="xp", bufs=1))
        wpool = ctx.enter_context(tc.tile_pool(name="wp", bufs=1))
        qk_pool = ctx.enter_context(tc.tile_pool(name="qk", bufs=1))
        vaug_pool = ctx.enter_context(tc.tile_pool(name="vaug", bufs=1))
        vt_pool = ctx.enter_context(tc.tile_pool(name="vtp", bufs=2))
        pt_pool = ctx.enter_context(tc.tile_pool(name="pt", bufs=3))
        ot_pool = ctx.enter_context(tc.tile_pool(name="otp", bufs=2))

        # weights + x DMAs, spread across queues, in consumption order
        xts = [[None] * NCP for _ in range(B)]
        wtile = {}
        wtile["q"] = wpool.tile([P, NCP, 2, DW], F8, tag="wq", name="wq")
        nc.sync.dma_start(out=wtile["q"][:], in_=wq_d)
        wtile["k"] = wpool.tile([P, NCP, 2, DW], F8, tag="wk", name="wk")
        nc.scalar.dma_start(out=wtile["k"][:], in_=wk_d)
        for b in range(B):
            for cp in range(NCP):
                t = xpool.tile([P, 2, T], F8, tag=f"x{b}_{cp}")
                eng = nc.sync if cp % 2 == 0 else nc.scalar
                eng.dma_start(out=t[:], in_=xT_d[b, cp])
                xts[b][cp] = t
        wtile["v"] = wpool.tile([P, NCP, 2, DW], F8, tag="wv", name="wv")
        nc.gpsimd.dma_start(out=wtile["v"][:], in_=wv_d)

        # warm the exp table during the DMA stall
        junk = const.tile([P, 1], FP)
        nc.vector.memset(junk[:], 0.0)
        junk2 = const.tile([P, 1], FP)
        nc.scalar.activation(junk2[:], junk[:], AF.Exp)

        negmask = const.tile([P, P], FP)
        nc.gpsimd.memset(negmask[:], 0.0)
        nc.gpsimd.affine_select(
            out=negmask[:], in_=negmask[:],
            compare_op=mybir.AluOpType.is_ge, fill=-1e30,
            base=0, pattern=[[1, P]], channel_multiplier=-1)
        ident = const.tile([P, P], BF)
        make_identity(nc, ident[:])
        ones_col = const.tile([P, HP, NT, 1], FP)
        nc.vector.memset(ones_col[:], 1.0)

        qts, kts, vaugs = [], [], []
        for b in range(B):
            qts.append(qk_pool.tile([P, T], BF, tag=f"qt{b}", name=f"qt{b}"))
            kts.append(qk_pool.tile([P, T], BF, tag=f"kt{b}", name=f"kt{b}"))
            # head-major so stripe pairs are adjacent: [P, HP, NT, DA]
            va = vaug_pool.tile([P, HP, NT, DA], BF, tag=f"va{b}",
                                name=f"va{b}")
            nc.vector.tensor_copy(va[:, :, :, DA - 1:DA], ones_col[:])
            vaugs.append(va)

        # ---- stage A(b0): q/k projections, cp-outer against the DMA stream
        b0, b1 = 0, 1
        with tc.tile_pool(name="qk_ps", bufs=1, space="PSUM") as qk_ps:
            q_ps = qk_ps.tile([P, T], FP, tag="qps", name="qps")
            k_ps = qk_ps.tile([P, T], FP, tag="kps", name="kps")
            for cp in range(NCP):
                for wname, ps in (("q", q_ps), ("k", k_ps)):
                    for n in range(T // 512):
                        nc.tensor.matmul(
                            ps[:, n * 512:(n + 1) * 512],
                            wtile[wname][:, cp, :, :],
                            xts[b0][cp][:, :, n * 512:(n + 1) * 512],
                            start=(cp == 0), stop=(cp == NCP - 1),
                            perf_mode=DR)
            for half in range(T // 1024):
                sl = slice(half * 1024, (half + 1) * 1024)
                nc.vector.tensor_copy(qts[b0][:, sl], q_ps[:, sl])
                nc.vector.tensor_copy(kts[b0][:, sl], k_ps[:, sl])

        # ---- filler quanta (run inside b0's stripe sections)
        fl_ctx = ExitStack()
        fl_ps = fl_ctx.enter_context(
            tc.tile_pool(name="fl_ps", bufs=1, space="PSUM"))

        def v_quant(b, n8):
            vps = fl_ps.tile([P, 512], FP, tag="proj", name="proj")
            for cp in range(NCP):
                nc.tensor.matmul(
                    vps[:], wtile["v"][:, cp, :, :],
                    xts[b][cp][:, :, n8 * 512:(n8 + 1) * 512],
                    start=(cp == 0), stop=(cp == NCP - 1), perf_mode=DR)
            vt = vt_pool.tile([P, 512], BF, tag="vt")
            nc.vector.tensor_copy(vt[:], vps[:])
            tp = fl_ps.tile([P, 4, P], BF, tag="tp", name="tp")
            for u in range(4):
                nc.tensor.transpose(
                    tp[:, u, :], vt[:, u * P:(u + 1) * P], ident[:])
            for h in range(HP):
                nc.vector.tensor_copy(
                    vaugs[b][:, h, n8 * 4:(n8 + 1) * 4, 0:DH],
                    tp[:, :, h * DH:(h + 1) * DH])

        def qk_quant(b, name, n8):
            dst = qts[b] if name == "q" else kts[b]
            ps = fl_ps.tile([P, 512], FP, tag="proj", name="proj")
            for cp in range(NCP):
                nc.tensor.matmul(
                    ps[:], wtile[name][:, cp, :, :],
                    xts[b][cp][:, :, n8 * 512:(n8 + 1) * 512],
                    start=(cp == 0), stop=(cp == NCP - 1), perf_mode=DR)
            nc.vector.tensor_copy(dst[:, n8 * 512:(n8 + 1) * 512], ps[:])

        v_quant(b0, 0)
        v_quant(b0, 1)
        filler = [lambda: v_quant(b0, 2), lambda: v_quant(b0, 3)]
        for n8 in (0, 1):
            filler.append(lambda n=n8: qk_quant(b1, "k", n))
        for n8 in (0, 1):
            filler.append(lambda n=n8: qk_quant(b1, "q", n))
        for n8 in (0, 1):
            filler.append(lambda n=n8: v_quant(b1, n))
        for n8 in (2, 3):
            filler.append(lambda n=n8: qk_quant(b1, "k", n))
        for n8 in (2, 3):
            filler.append(lambda n=n8: qk_quant(b1, "q", n))
        for n8 in (2, 3):
            filler.append(lambda n=n8: v_quant(b1, n))

        def chunks_for(a0, q_hi):
            out = []
            for m in range(a0 // 512, q_hi // 512):
                out.append((max(a0, m * 512), (m + 1) * 512))
            return out

        def section(b, h, kq, s_ps, o_ps, use_filler):
            q_lo, q_hi = 1024 * kq, 1024 * (kq + 1)
            hs = slice(h * DH, (h + 1) * DH)
            o_t = o_ps.tile([DA, 1024], FP, tag="o", name="o")
            pend = None

            def emit_pv(ent):
                j, chunks, ptk = ent
                va = vaugs[b][:, h, j, :]
                for (a, e) in chunks:
                    nc.tensor.matmul(
                        o_t[:, a - q_lo:e - q_lo],
                        va, ptk[:, a - q_lo:e - q_lo],
                        start=(j == 0), stop=(j == e // P - 1))

            for j in range(8 * (kq + 1)):
                s0 = j * P
                a0 = max(s0, q_lo)
                chunks = chunks_for(a0, q_hi)
                stl = s_ps.tile([P, 1024], FP, tag="s", name="s")
                for (a, e) in chunks:
                    nc.tensor.matmul(
                        stl[:, a - q_lo:e - q_lo],
                        kts[b][hs, s0:s0 + P], qts[b][hs, a:e],
                        start=True, stop=True)
                if q_lo <= s0:
                    nc.vector.tensor_add(
                        stl[:, s0 - q_lo:s0 - q_lo + P],
                        stl[:, s0 - q_lo:s0 - q_lo + P], negmask[:])
                ptk = pt_pool.tile([P, 1024], BF, tag="ptk")
                nc.scalar.activation(
                    ptk[:, a0 - q_lo:1024], stl[:, a0 - q_lo:1024],
                    AF.Exp, scale=scale)
                if pend is not None:
                    emit_pv(pend)
                pend = (j, chunks, ptk)
                if use_filler and j % 2 == 0 and filler:
                    filler.pop(0)()
            emit_pv(pend)
            osb = ot_pool.tile([DA, 1024], BF, tag="osb")
            nc.vector.tensor_copy(osb[:], o_t[:])
            nc.gpsimd.dma_start(out=ot_d[b, h, :, q_lo:q_hi], in_=osb[:])

        with tc.tile_pool(name="s_ps", bufs=2, space="PSUM") as s_ps, \
             tc.tile_pool(name="o_ps", bufs=1, space="PSUM") as o_ps:
            for (h, kq) in ((0, 0), (0, 1), (1, 0), (1, 1)):
                section(b0, h, kq, s_ps, o_ps, True)
        while filler:
            filler.pop(0)()
        fl_ctx.close()
        with tc.tile_pool(name="s_ps1", bufs=3, space="PSUM") as s_ps1, \
             tc.tile_pool(name="o_ps1", bufs=1, space="PSUM") as o_ps1:
            for (h, kq) in ((0, 0), (0, 1), (1, 0), (1, 1)):
                section(b1, h, kq, s_ps1, o_ps1, False)
    nc.compile()
    return nc


# --------------------------------------------------------------------------
# phase 2: per-core Wo projection + residual + rmsnorm + FFN + rmsnorm
# --------------------------------------------------------------------------

def build_phase2(NTOK, C, DFF):
    NTB = NTOK // P             # 4 token blocks
    NCH = C // P                # 8 channel chunks
    NDF = DFF // P              # 32 dff chunks
    NG = DFF // 512             # 8 w1 groups
    STAG = 12                   # staggered tail d-chunks per token block

    nc = bacc.Bacc("TRN2", debug=False)
    # host-packed layouts (contiguous per DMA)
    xc_d = nc.dram_tensor("xc", [P, NTB * C], BF, kind="ExternalInput").ap()
    at_d = nc.dram_tensor("attnT", [2, P, 4, NTOK], F8,
                          kind="ExternalInput").ap()
    wo_d = nc.dram_tensor("wo", [2, P, 4, C], F8, kind="ExternalInput").ap()
    w1_d = nc.dram_tensor("w1", [NG, P, NCH * 512], BF,
                          kind="ExternalInput").ap()
    w2_d = nc.dram_tensor("w2", [NDF // 4, P, 4 * C], BF,
                          kind="ExternalInput").ap()
    g1_d = nc.dram_tensor("g1", [C], FP, kind="ExternalInput").ap()
    g2_d = nc.dram_tensor("g2", [C], FP, kind="ExternalInput").ap()
    b1_d = nc.dram_tensor("b1", [DFF], FP, kind="ExternalInput").ap()
    b2_d = nc.dram_tensor("b2", [C], BF, kind="ExternalInput").ap()
    out_d = nc.dram_tensor("out", [NTOK, C], BF, kind="ExternalOutput").ap()

    def bcast_rows(src_ap, cols):
        return bass.AP(tensor=src_ap.tensor, offset=src_ap.offset,
                       ap=[[0, P], [1, cols]])

    halves = ((0, 512), (512, 512))

    with tile.TileContext(nc) as tc, ExitStack() as ctx:
        const = ctx.enter_context(tc.tile_pool(name="const", bufs=1))
        work = ctx.enter_context(tc.tile_pool(name="work", bufs=2))
        stats = ctx.enter_context(tc.tile_pool(name="stats", bufs=4))
        h_pool = ctx.enter_context(tc.tile_pool(name="hp", bufs=1))
        at_pool = ctx.enter_context(tc.tile_pool(name="atp", bufs=1))
        out_pool = ctx.enter_context(tc.tile_pool(name="outp", bufs=2))
        wo_po = ctx.enter_context(tc.tile_pool(name="wop", bufs=1))

        # input DMAs: att/w1 on sync, wo/xc/w2 on scalar, vectors on vector
        att_t, wot_t = [], []
        for g in range(2):
            at4 = wo_po.tile([P, 4, NTOK], F8, tag=f"at{g}", name=f"at{g}")
            nc.sync.dma_start(out=at4[:], in_=at_d[g])
            att_t.append(at4)
            wo4 = wo_po.tile([P, 4, C], F8, tag=f"wo{g}", name=f"wo{g}")
            nc.scalar.dma_start(out=wo4[:], in_=wo_d[g])
            wot_t.append(wo4)
        xc_t = wo_po.tile([P, NTB * C], BF, tag="xc", name="xc")
        nc.scalar.dma_start(out=xc_t[:], in_=xc_d)

        eps_t = const.tile([P, 1], FP)
        nc.vector.memset(eps_t[:], EPS)
        g1b = const.tile([P, C], FP)
        nc.gpsimd.dma_start(out=g1b[:], in_=bcast_rows(g1_d, C))
        g2b = const.tile([P, C], FP)
        nc.gpsimd.dma_start(out=g2b[:], in_=bcast_rows(g2_d, C))
        b2b = const.tile([P, C], BF)
        nc.gpsimd.dma_start(out=b2b[:], in_=bcast_rows(b2_d, C))
        b1s = const.tile([P, NDF], FP)
        nc.gpsimd.dma_start(
            out=b1s[:],
            in_=bass.AP(tensor=b1_d.tensor, offset=b1_d.offset,
                        ap=[[1, P], [P, NDF]]))
        # w1 groups (sync queue), 3-deep window
        w1_po = ctx.enter_context(tc.tile_pool(name="w1p", bufs=3))
        w1g = []
        for g in range(NG):
            t = w1_po.tile([P, NCH * 512], BF, tag="w1g", name="w1g")
            nc.sync.dma_start(out=t[:], in_=w1_d[g])
            w1g.append(t)
        # w2 quads (scalar queue), 3-deep window
        w2_po = ctx.enter_context(tc.tile_pool(name="w2p", bufs=3))
        w2q = []
        for q in range(NDF // 4):
            t = w2_po.tile([P, 4 * C], BF, tag="w2q", name="w2q")
            nc.scalar.dma_start(out=t[:], in_=w2_d[q])
            w2q.append(t)

        def w2c(d, lo, hi):
            base = (d % 4) * C
            return w2q[d // 4][:, base + lo:base + hi]

        junk = const.tile([P, 1], FP)
        nc.vector.memset(junk[:], 1.0)
        junk2 = const.tile([P, 1], FP)
        nc.scalar.activation(junk2[:], junk[:], AF.Sqrt)
        ident = const.tile([P, P], BF)
        make_identity(nc, ident[:])

        hT = h_pool.tile([P, NCH, NTOK], BF, tag="hT")
        h_bfs = []

        # ---- stage 0: o = attn@Wo + x; rmsnorm per token block; -> hT
        with tc.tile_pool(name="o_ps", bufs=1, space="PSUM") as o_ps, \
             tc.tile_pool(name="t_ps", bufs=2, space="PSUM") as t_ps:
            for pair_i in range(NTB // 2):
                pair = (2 * pair_i, 2 * pair_i + 1)
                # 3 rotating tag slots so a fresh pair never waits on the
                # slowest token block of the previous pair
                sl_ids = [(2 * pair_i) % 3, (2 * pair_i + 1) % 3]
                o2 = [[o_ps.tile([P, 512], FP, tag=f"o2_{sl_ids[si]}_{hi}",
                                 name="o2")
                       for hi in range(2)] for si in range(2)]
                for cp in range(NCH // 2):
                    g, jj = cp // 2, (cp % 2) * 2
                    for si, tb in enumerate(pair):
                        for hi, (hst, hw) in enumerate(halves):
                            nc.tensor.matmul(
                                o2[si][hi][:],
                                att_t[g][:, jj:jj + 2, tb * P:(tb + 1) * P],
                                wot_t[g][:, jj:jj + 2, hst:hst + hw],
                                start=(cp == 0), stop=False,
                                perf_mode=mybir.MatmulPerfMode.DoubleRow)
                for si, tb in enumerate(pair):
                    for hi, (hst, hw) in enumerate(halves):
                        nc.tensor.matmul(
                            o2[si][hi][:], ident[:],
                            xc_t[:, tb * C + hst:tb * C + hst + hw],
                            start=False, stop=True)
                for si, tb in enumerate(pair):
                    sq = work.tile([P, 512], FP, tag="sq")
                    ss = [stats.tile([P, 1], FP, tag=f"ss{hi}",
                                     name=f"ss{hi}") for hi in range(2)]
                    for hi in range(2):
                        nc.scalar.activation(sq[:], o2[si][hi][:], AF.Square,
                                             accum_out=ss[hi][:])
                    nc.vector.tensor_add(ss[0][:], ss[0][:], ss[1][:])
                    rstd = stats.tile([P, 1], FP, tag="rstd")
                    nc.scalar.activation(rstd[:], ss[0][:], AF.Sqrt,
                                         scale=1.0 / C, bias=eps_t[:])
                    rinv = stats.tile([P, 1], FP, tag="rinv")
                    nc.vector.reciprocal(rinv[:], rstd[:])
                    h_bf = h_pool.tile([P, C], BF, tag=f"h{tb}")
                    for hi, (hst, hw) in enumerate(halves):
                        nc.vector.scalar_tensor_tensor(
                            h_bf[:, hst:hst + hw], o2[si][hi][:], rinv[:],
                            g1b[:, hst:hst + hw],
                            op0=mybir.AluOpType.mult, op1=mybir.AluOpType.mult)
                    h_bfs.append(h_bf)
                    for g4 in range(2):
                        tp = t_ps.tile([P, 4, P], BF, tag="tp")
                        for u in range(4):
                            nc.tensor.transpose(
                                tp[:, u, :],
                                h_bf[:, (g4 * 4 + u) * P:(g4 * 4 + u + 1) * P],
                                ident[:])
                        nc.vector.tensor_copy(
                            hT[:, g4 * 4:(g4 + 1) * 4, tb * P:(tb + 1) * P],
                            tp[:])

        # ---- stage 1: aT = silu(W1^T @ h^T + b1) via scalar-engine silu
        ats = []
        hb2s = []
        with tc.tile_pool(name="a_ps", bufs=2, space="PSUM") as a_ps:
            for g in range(NG):
                aps = [a_ps.tile([P, NTOK], FP, tag=f"a{u}", name=f"a{u}")
                       for u in range(4)]
                for c in range(NCH):
                    for u in range(4):
                        nc.tensor.matmul(
                            aps[u][:],
                            w1g[g][:, c * 512 + u * P:c * 512 + (u + 1) * P],
                            hT[:, c, :],
                            start=(c == 0), stop=(c == NCH - 1))
                for u in range(4):
                    d = 4 * g + u
                    at_t = at_pool.tile([P, NTOK], BF, tag=f"at{d}")
                    nc.scalar.activation(at_t[:], aps[u][:], AF.Silu,
                                         bias=b1s[:, d:d + 1], scale=1.0)
                    ats.append(at_t)
                if g == 0:
                    # DVE is idle here: precompute h + b2 for stage 2
                    for tb in range(NTB):
                        hb2 = h_pool.tile([P, C], BF, tag=f"hb2_{tb}",
                                          name="hb2")
                        nc.vector.tensor_add(hb2[:], h_bfs[tb][:], b2b[:])
                        hb2s.append(hb2)

        # ---- stage 2: f = aT^T @ W2 + h + b2; rmsnorm + store per block
        with tc.tile_pool(name="f_ps", bufs=1, space="PSUM") as f_ps:
            f2 = [f_ps.tile([P, C], FP, tag=f"f{tb}", name=f"f{tb}")
                  for tb in range(NTB)]
            for tb in range(NTB):
                for (hst, hw) in halves:
                    nc.tensor.matmul(
                        f2[tb][:, hst:hst + hw], ident[:],
                        hb2s[tb][:, hst:hst + hw],
                        start=True, stop=False)
            for d in range(NDF - STAG):
                for tb in range(NTB):
                    for (hst, hw) in halves:
                        nc.tensor.matmul(
                            f2[tb][:, hst:hst + hw],
                            ats[d][:, tb * P:(tb + 1) * P],
                            w2c(d, hst, hst + hw),
                            start=False, stop=False)
            for tb in range(NTB):
                for d in range(NDF - STAG, NDF):
                    for (hst, hw) in halves:
                        nc.tensor.matmul(
                            f2[tb][:, hst:hst + hw],
                            ats[d][:, tb * P:(tb + 1) * P],
                            w2c(d, hst, hst + hw),
                            start=False, stop=(d == NDF - 1))
                sq = work.tile([P, C], FP, tag="sq2")
                ssum = stats.tile([P, 1], FP, tag="ssum2")
                nc.scalar.activation(sq[:], f2[tb][:], AF.Square,
                                     accum_out=ssum[:])
                rstd = stats.tile([P, 1], FP, tag="rstd2")
                nc.scalar.activation(rstd[:], ssum[:], AF.Sqrt,
                                     scale=1.0 / C, bias=eps_t[:])
                rinv = stats.tile([P, 1], FP, tag="rinv2")
                nc.vector.reciprocal(rinv[:], rstd[:])
                o = out_pool.tile([P, C], BF, tag="outt")
                nc.vector.scalar_tensor_tensor(
                    o[:], f2[tb][:], rinv[:], g2b[:],
                    op0=mybir.AluOpType.mult, op1=mybir.AluOpType.mult)
                nc.gpsimd.dma_start(
                    out=out_d[tb * P:(tb + 1) * P, :], in_=o[:])
    nc.compile()
    return nc


# --------------------------------------------------------------------------
# host orchestration
# --------------------------------------------------------------------------

_CACHE = {}


def _phase1(B, T, C, DH):
    key = ("p1", B, T, C, DH)
    if key not in _CACHE:
        _CACHE[key] = build_phase1(B, T, C, DH)
    return _CACHE[key]


def _phase2(NTOK, C, DFF):
    key = ("p2", NTOK, C, DFF)
    if key not in _CACHE:
        _CACHE[key] = build_phase2(NTOK, C, DFF)
    return _CACHE[key]


def _run(nc, in_maps):
    import os
    trace = bool(os.environ.get("KERNEL_TRACE"))
    res = run_bass_kernel_spmd(nc, in_maps, core_ids=list(range(N_CORES)),
                               trace=trace)
    LAST_EXEC_NS.append(res.exec_time_ns)
    LAST_TRACES.append(res.instructions_and_trace)
    return res.results


def _pack_rows(a, nrow):
    """[R, W] -> [P, (R//P//nrow groups)...]: group rows so each DMA tile
    [P, nrow*W] is contiguous: out[g, p, i, :] = a[(g*nrow+i)*P + p, :]."""
    R, W = a.shape
    ng = R // (P * nrow)
    return np.ascontiguousarray(
        a.reshape(ng, nrow, P, W).transpose(0, 2, 1, 3).reshape(
            ng, P, nrow * W))


def kernel(x, Wq, Wk, Wv, Wo, bo, W1, b1, W2, b2, g1, g2):
    f32 = lambda a: np.ascontiguousarray(np.asarray(a), dtype=np.float32)
    x = f32(x)
    Wq, Wk, Wv, Wo, bo = f32(Wq), f32(Wk), f32(Wv), f32(Wo), f32(bo)
    W1, b1, W2, b2, g1, g2 = f32(W1), f32(b1), f32(W2), f32(b2), f32(g1), f32(g2)

    B, T, C = x.shape
    H, _, DH = Wq.shape
    HP = H // N_CORES
    DA = DH + 1
    NCC = C // P
    DW = HP * DH
    LAST_EXEC_NS.clear()
    LAST_TRACES.clear()

    # ---- phase 1
    nc1 = _phase1(B, T, C, DH)
    NCP = NCC // 2
    xT = x.transpose(0, 2, 1).astype(F8_NP)            # [B, C, T] fp8
    # pack x: [B, NCP, P, 2, T] with (b,cp,p,i,t) = xT[b, (2cp+i)P+p, t]
    xP = np.ascontiguousarray(
        xT.reshape(B, NCP, 2, P, T).transpose(0, 1, 3, 2, 4))
    in1 = []
    for i in range(N_CORES):
        ws = {}
        for nm, W_ in (("wq", Wq), ("wk", Wk), ("wv", Wv)):
            pw = W_[HP * i:HP * (i + 1)].transpose(1, 0, 2).reshape(C, DW)
            # x32 for fp8 range; [P, NCP, 2, DW], (p,cp,i,m)=pw[(2cp+i)P+p,m]
            ws[nm] = np.ascontiguousarray(
                (pw * 32.0).astype(F8_NP)
                .reshape(NCP, 2, P, DW).transpose(2, 0, 1, 3))
        in1.append({"xT": xP, **ws})
    res1 = _run(nc1, in1)

    attn = np.empty((B, T, C), np.float32)
    for i in range(N_CORES):
        ot = res1[i]["ot"].astype(np.float32)          # [B, HP, DA, T]
        o = ot[:, :, :DH, :]
        den = ot[:, :, DH, :]
        on = o / (32.0 * den[:, :, None, :])           # undo v x32 scale
        for hh in range(HP):
            hcol = (HP * i + hh) * DH
            attn[:, :, hcol:hcol + DH] = on[:, hh].transpose(0, 2, 1)

    # ---- phase 2
    NTOK = B * T // N_CORES
    DFF = W1.shape[1]
    NTB = NTOK // P
    nc2 = _phase2(NTOK, C, DFF)
    # rmsnorm is scale-invariant: attnT x8 and wo x32 go into fp8 range,
    # and the residual x picks up the matching x256
    xf = ((x.reshape(B * T, C) + bo) * 256.0).astype(BF_NP)
    af = attn.reshape(B * T, C) * 8.0
    NCH = C // P
    NG = DFF // 512
    # w1P[g][p, c*512+f] = W1[c*128+p, g*512+f]
    w1P = np.ascontiguousarray(
        W1.astype(BF_NP).reshape(NCH, P, NG, 512).transpose(2, 1, 0, 3)
        .reshape(NG, P, NCH * 512))
    w2P = _pack_rows(W2.astype(BF_NP), 4)              # [8, P, 4*C]
    in2 = []
    for k in range(N_CORES):
        sl = slice(k * NTOK, (k + 1) * NTOK)
        atT = np.ascontiguousarray(af[sl].T).astype(F8_NP)   # [C, NTOK]
        in2.append({
            "xc": _pack_rows(xf[sl], NTB)[0],
            "attnT": _pack_rows(atT, 4).reshape(2, P, 4, NTOK),
            "wo": _pack_rows((Wo * 32.0).astype(F8_NP), 4)
                  .reshape(2, P, 4, C),
            "w1": w1P, "w2": w2P,
            "g1": g1, "g2": g2, "b1": b1, "b2": b2.astype(BF_NP),
        })
    res2 = _run(nc2, in2)
    out = np.concatenate(
        [res2[k]["out"].astype(np.float32) for k in range(N_CORES)], axis=0)
    return out.reshape(B, T, C)
